# revision 38
# baseline (speedup 1.0000x reference)
"""Bass/Tile Trainium2 kernel for the additive-attention module.

reference (per batch row b):
    q = hidden_state @ Wa.T + ba                 # [A]
    k = feature_vectors[b] @ Ua.T                # [L, A]
    e = tanh(q + k) @ w                          # [L]
    attn = softmax(e)                            # [L]
    context[b] = attn @ feature_vectors[b]       # [M]

Sharding: data-parallel over batch B=64 -> 8 cores x 8 rows, params
replicated, no collectives. Each core streams its 32 MB feature_vector
shard from HBM exactly once.

Precision: fp32 matmuls cost 4 cycles/column on TRN2 PE; fp16 costs 1.
The fv pipeline (fv, Ua, tanh output, attn weights) runs in fp16;
softmax statistics and all accumulations (PSUM) stay fp32.

Per-core dataflow (per batch row):
  - fv cast fp32->fp16 during the HBM DMA (SWDGE), natural [l, m] layout
  - PE transposes 128x128 fp16 tiles of fv into [m, l] layout (PSUM),
    DVE evacuates; k-matmul streams fv.T columns with Ua.T stationary
    (XBAR dma transpose measured 16us/row + full serialization against
    the SWDGE loads, so the PE identity-transpose is the right path)
  - ScalarE evacuates the k PSUM with fused per-partition bias q[a] and
    tanh in one ACTIVATE (fp16 out)
  - e = w.T @ tanh(...) on PE; softmax: DRAM-bounce reshape of e
    [1,4096] -> [128,32], DVE row max, GPSIMD cross-partition max, ACT
    exp with accum_out row sums, GPSIMD cross-partition sum
  - weighted sum on PE: attn column [128,1] fp16 stationary, natural
    fv tiles streaming; denominator applied at the end in fp32
  - queue discipline: fv loads get the GPSIMD SWDGE queue (with the
    cross-partition reduces, emitted only when nearly ready), e bounce +
    ctx stores ride Sync, param prologue rides the Scalar HWDGE queue
  - row b's softmax tail is split and deferred into row b+1's emission
    (gather+max at the 3rd k-group, exp at the very end) and its
    weighted sum after row b+1's main stage, so neither PE nor any
    in-order queue ever waits on the softmax dependency chain
"""

import numpy as np

B, R, M, A, L = 64, 512, 256, 256, 4096
NCORES = 8
BLOC = B // NCORES  # 8 batch rows per core
NL = L // 128  # 32 l-chunks of 128
NJG = 8  # j-groups of 512 l-columns
JW = L // NJG  # 512

_CACHE = {}


def _build():
    from contextlib import ExitStack

    import concourse.bacc as bacc
    import concourse.bass as bass
    import concourse.bass_isa as bass_isa
    import concourse.mybir as mybir
    import concourse.tile as tile
    from concourse.masks import make_identity

    f32 = mybir.dt.float32
    f16 = mybir.dt.float16
    AF = mybir.ActivationFunctionType

    nc = bacc.Bacc("TRN2", target_bir_lowering=False, debug=False,
                   num_devices=NCORES)

    hs = nc.dram_tensor("hidden_state", [BLOC, R], f32, kind="ExternalInput").ap()
    fv = nc.dram_tensor("feature_vectors", [BLOC, L, M], f32,
                        kind="ExternalInput").ap()
    Wa = nc.dram_tensor("Wa", [A, R], f32, kind="ExternalInput").ap()
    Ua = nc.dram_tensor("Ua", [A, M], f32, kind="ExternalInput").ap()
    w = nc.dram_tensor("w", [A, 1], f32, kind="ExternalInput").ap()
    ba = nc.dram_tensor("ba", [1, A], f32, kind="ExternalInput").ap()
    ctx_out = nc.dram_tensor("context", [BLOC, M], f32, kind="ExternalOutput").ap()

    with tile.TileContext(nc) as tc, ExitStack() as ctx:
        singles = ctx.enter_context(tc.tile_pool(name="singles", bufs=1))
        ldpool = ctx.enter_context(tc.tile_pool(name="ldpool", bufs=2))
        fvpool = ctx.enter_context(tc.tile_pool(name="fvpool", bufs=3))
        work = ctx.enter_context(tc.tile_pool(name="work", bufs=3))
        small = ctx.enter_context(tc.tile_pool(name="small", bufs=2))
        ps_tp = ctx.enter_context(tc.tile_pool(name="ps_tp", bufs=2, space="PSUM"))
        ps_k = ctx.enter_context(tc.tile_pool(name="ps_k", bufs=3, space="PSUM"))
        ps_e = ctx.enter_context(tc.tile_pool(name="ps_e", bufs=2, space="PSUM"))
        ps_mm = ctx.enter_context(tc.tile_pool(name="ps_mm", bufs=1, space="PSUM"))
        dram = ctx.enter_context(tc.tile_pool(name="dram", bufs=2, space="DRAM"))

        # identities first: they ride the GPSIMD queue and gate every
        # transpose, so they must precede the fv load issues there
        ident = singles.tile([128, 128], f32, tag="ident", name="ident")
        make_identity(nc, ident)
        ident16 = singles.tile([128, 128], f16, tag="ident16", name="ident16")
        make_identity(nc, ident16)

        # batch 0's fv cast-load is issued split in four so the prologue
        # transposes chase the arriving 1 MB chunks; steady-state loads
        # stay whole
        fv_nat0 = fvpool.tile([128, NL, M], f16, tag="fv", name="fv")
        for part in range(4):
            fsrc = bass.AP(tensor=fv.tensor, offset=part * (NL // 4) * 128 * M,
                           ap=[[M, 128], [128 * M, NL // 4], [1, M]])
            nc.gpsimd.dma_start(
                out=fv_nat0[:, part * (NL // 4):(part + 1) * (NL // 4), :],
                in_=fsrc)

        # ---- parameters into contraction-major layouts ----
        # (param DMAs ride the Scalar HWDGE queue; Sync keeps the e bounce,
        # GPSIMD keeps the fv loads)
        # WaT[rt] [128(r), 256(a)] fp32: WaT[rt][k, a] = Wa[a, 128*rt + k]
        WaT = [singles.tile([128, A], f32, tag=f"WaT{rt}", name=f"WaT{rt}")
               for rt in range(4)]
        for at in range(2):
            wa_nat = ldpool.tile([128, R], f32, tag="ld", name="ld")
            nc.scalar.dma_start(out=wa_nat, in_=Wa[at * 128:(at + 1) * 128, :])
            for rt in range(4):
                ps = ps_mm.tile([128, 128], f32, tag="mm", name="mm")
                nc.tensor.transpose(ps, wa_nat[:, rt * 128:(rt + 1) * 128], ident)
                nc.vector.tensor_copy(out=WaT[rt][:, at * 128:(at + 1) * 128],
                                      in_=ps)
        # UaT[mh] [128(m), 256(a)] fp16: UaT[mh][k, a] = Ua[a, 128*mh + k]
        UaT = [singles.tile([128, A], f16, tag=f"UaT{mh}", name=f"UaT{mh}")
               for mh in range(2)]
        for at in range(2):
            ua_nat = ldpool.tile([128, M], f32, tag="ld", name="ld")
            nc.scalar.dma_start(out=ua_nat, in_=Ua[at * 128:(at + 1) * 128, :])
            for mh in range(2):
                ps = ps_mm.tile([128, 128], f32, tag="mm", name="mm")
                nc.tensor.transpose(ps, ua_nat[:, mh * 128:(mh + 1) * 128], ident)
                nc.vector.tensor_copy(out=UaT[mh][:, at * 128:(at + 1) * 128],
                                      in_=ps)
        # w as fp16 stationary columns [128, 1] per a-half (cast during DMA)
        w_sb = [singles.tile([128, 1], f16, tag=f"w{ah}", name=f"w{ah}")
                for ah in range(2)]
        for ah in range(2):
            nc.gpsimd.dma_start(out=w_sb[ah], in_=w[ah * 128:(ah + 1) * 128, :])

        # hsT[rt] [128(r), BLOC] fp32 — the strided gather rides the GPSIMD
        # SWDGE queue (cheap descriptor gen there; on the Scalar HWDGE
        # queue it would block ba/ua for ~14us)
        hsT = [singles.tile([128, BLOC], f32, tag=f"hsT{rt}", name=f"hsT{rt}")
               for rt in range(4)]
        for rt in range(4):
            src = bass.AP(tensor=hs.tensor, offset=rt * 128,
                          ap=[[1, 128], [R, BLOC]])
            nc.gpsimd.dma_start(out=hsT[rt], in_=src)

        # q = hs @ Wa.T + ba   -> [BLOC, A] fp32
        ba_b = singles.tile([BLOC, A], f32, tag="ba", name="ba")
        nc.scalar.dma_start(out=ba_b,
                            in_=bass.AP(tensor=ba.tensor, offset=0,
                                        ap=[[0, BLOC], [1, A]]))
        q_ps = ps_mm.tile([BLOC, A], f32, tag="mm", name="mm")
        for rt in range(4):
            nc.tensor.matmul(q_ps, lhsT=hsT[rt], rhs=WaT[rt],
                             start=(rt == 0), stop=(rt == 3))
        q_sb = singles.tile([BLOC, A], f32, tag="q", name="q")
        nc.vector.tensor_add(q_sb, q_ps, ba_b)
        # qT[ah] [128(a), BLOC] fp32
        qT = [singles.tile([128, BLOC], f32, tag=f"qT{ah}", name=f"qT{ah}")
              for ah in range(2)]
        for ah in range(2):
            ps = ps_mm.tile([128, BLOC], f32, tag="mm", name="mm")
            nc.tensor.transpose(ps, q_sb[:, ah * 128:(ah + 1) * 128],
                                ident[:BLOC, :BLOC])
            nc.vector.tensor_copy(out=qT[ah], in_=ps)

        # ---- deferred softmax tail machinery ----
        state = {}  # b -> dict with per-row tiles awaiting tail pieces
        done = {}   # b -> (fv_nat, p_t, rz) awaiting weighted sum

        def tail_gather(b):
            """e bounce-back DMA for row b (emitted early in row b+1: the
            e_d writes of row b are complete by then). The max reduction
            is deferred to tail_max so a late gather never blocks the
            in-order DVE queue in front of the fvT evacuation copies."""
            st = state[b]
            e_t = small.tile([128, NL], f32, tag="e_t", name="e_t")
            nc.sync.dma_start(
                out=e_t,
                in_=bass.AP(tensor=st["e_d"].tensor, offset=st["e_d"].offset,
                            ap=[[NL, 128], [1, NL]]))
            st["e_t"] = e_t

        def tail_max(b):
            st = state[b]
            mrow = small.tile([128, 1], f32, tag="mrow", name="mrow")
            nc.vector.reduce_max(out=mrow, in_=st["e_t"],
                                 axis=mybir.AxisListType.X)
            mall = small.tile([128, 1], f32, tag="mall", name="mall")
            nc.gpsimd.partition_all_reduce(mall, mrow, channels=128,
                                           reduce_op=bass_isa.ReduceOp.max)
            negm = small.tile([128, 1], f32, tag="negm", name="negm")
            nc.vector.tensor_scalar_mul(negm, mall, -1.0)
            st["negm"] = negm

        def tail_exp(b):
            """exp + sum for row b (emitted at the end of row b+1's main
            stage so it never blocks row b+1's tanh on the ACT queue)."""
            st = state.pop(b)
            p_t = small.tile([128, NL], f16, tag="p_t", name="p_t")
            srow = small.tile([128, 1], f32, tag="srow", name="srow")
            nc.scalar.activation(out=p_t, in_=st["e_t"], func=AF.Exp,
                                 bias=st["negm"], scale=1.0, accum_out=srow)
            sall = small.tile([128, 1], f32, tag="sall", name="sall")
            nc.gpsimd.partition_all_reduce(sall, srow, channels=128,
                                           reduce_op=bass_isa.ReduceOp.add)
            rz = small.tile([1, 1], f32, tag="rz", name="rz")
            nc.vector.reciprocal(out=rz, in_=sall[0:1, :])
            done[b] = (st["fv_nat"], p_t, rz)

        # ---- main per-batch-row pipeline ----
        def main_stage(b, fv_nat):
            # next row's load is issued first so the GPSIMD queue never
            # sits behind the cross-partition reduces
            if b + 1 < BLOC:
                fv_nat_n = fvpool.tile([128, NL, M], f16, tag="fv", name="fv")
                src = bass.AP(tensor=fv.tensor, offset=(b + 1) * L * M,
                              ap=[[M, 128], [128 * M, NL], [1, M]])
                nc.gpsimd.dma_start(out=fv_nat_n, in_=src)
            else:
                fv_nat_n = None

            e_sb = small.tile([1, L], f32, tag="e_sb", name="e_sb")
            e_d = dram.tile([L], f32, tag="e_d", name="e_d")

            # software-pipelined j-groups: PE emits transposes(i),
            # k-matmuls(i-1), e-matmul(i-2) per step so it never stalls on
            # the DVE fvT-copy or ACT tanh of the current group.
            fvT_q = {}
            t_q = {}

            def emit_T(jg):
                # both m-halves' transposes share one fp16 PSUM bank;
                # alternate the mh target region between consecutive
                # transposes so their weight loads can pipeline
                pst = ps_tp.tile([128, 2, JW], f16, tag="tp", name="tp")
                for c in range(4):
                    t = jg * 4 + c
                    for mh in range(2):
                        nc.tensor.transpose(
                            pst[:, mh, c * 128:(c + 1) * 128],
                            fv_nat[:, t, mh * 128:(mh + 1) * 128], ident16)
                fvT = [work.tile([128, JW], f16, tag=f"fvT{mh}", name=f"fvT{mh}")
                       for mh in range(2)]
                for mh in range(2):
                    nc.vector.tensor_copy(out=fvT[mh], in_=pst[:, mh, :])
                fvT_q[jg] = fvT

            def emit_K(jg):
                # the two ah accumulation groups are interleaved so each
                # matmul's weight load overlaps the other group's stream
                # (within one group the mh0->mh1 pair is strictly serial)
                fvT = fvT_q.pop(jg)
                t_sb = [work.tile([128, JW], f16, tag=f"t{ah}", name=f"t{ah}")
                        for ah in range(2)]
                psk = [ps_k.tile([128, JW], f32, tag="kk", name="kk")
                       for _ in range(2)]
                for mh in range(2):
                    for ah in range(2):
                        nc.tensor.matmul(
                            psk[ah], lhsT=UaT[mh][:, ah * 128:(ah + 1) * 128],
                            rhs=fvT[mh], start=(mh == 0), stop=(mh == 1))
                for ah in range(2):
                    nc.scalar.activation(out=t_sb[ah], in_=psk[ah],
                                         func=AF.Tanh,
                                         bias=qT[ah][:, b:b + 1], scale=1.0)
                t_q[jg] = t_sb

            def emit_E(jg):
                t_sb = t_q.pop(jg)
                pse = ps_e.tile([1, JW], f32, tag="ee", name="ee")
                for ah in range(2):
                    nc.tensor.matmul(pse, lhsT=w_sb[ah], rhs=t_sb[ah],
                                     start=(ah == 0), stop=(ah == 1))
                nc.vector.tensor_copy(out=e_sb[:, jg * JW:(jg + 1) * JW],
                                      in_=pse)
                # scatter into partition-major e_d[p*32 + t] so the
                # latency-critical gather reads 128B-contiguous runs;
                # the 512 4-byte write descriptors are pipelined and
                # off the critical path
                nc.sync.dma_start(
                    out=bass.AP(tensor=e_d.tensor, offset=e_d.offset + 4 * jg,
                                ap=[[0, 1], [1, 4], [NL, 128]]),
                    in_=e_sb[:, jg * JW:(jg + 1) * JW].rearrange(
                        "p (c q) -> p c q", c=4))

            part = {}
            for i in range(NJG + 2):
                if i < NJG:
                    emit_T(i)
                if i == 3 and (b - 1) in state:
                    tail_gather(b - 1)
                if 1 <= i <= NJG:
                    emit_K(i - 1)
                if 2 <= i:
                    emit_E(i - 2)
                if b == BLOC - 1 and i == NJG - 1:
                    # final row: gather + partial max over chunks 0..23
                    # while j-groups 6..7 still compute, shortening the
                    # end-of-kernel softmax chain
                    e_t = small.tile([128, NL], f32, tag="e_t", name="e_t")
                    nc.sync.dma_start(
                        out=e_t[:, 0:24],
                        in_=bass.AP(tensor=e_d.tensor, offset=e_d.offset,
                                    ap=[[NL, 128], [1, 24]]))
                    mrow1 = small.tile([128, 1], f32, tag="mall",
                                       name="mrow1")
                    nc.vector.reduce_max(out=mrow1, in_=e_t[:, 0:24],
                                         axis=mybir.AxisListType.X)
                    part = {"e_t": e_t, "mrow1": mrow1}

            state[b] = {"fv_nat": fv_nat, "e_d": e_d, **part}
            if (b - 1) in state:
                tail_max(b - 1)
                tail_exp(b - 1)
            return fv_nat_n

        # weighted sum, one batch row behind
        def ws_stage(b):
            fv_nat, p_t, rz = done.pop(b)
            psw = ps_mm.tile([1, M], f32, tag="mm", name="mm")
            for t in range(NL):
                nc.tensor.matmul(psw, lhsT=p_t[:, t:t + 1], rhs=fv_nat[:, t, :],
                                 start=(t == 0), stop=(t == NL - 1))
            ctxs = small.tile([1, M], f32, tag="ctx", name="ctx")
            nc.vector.tensor_scalar_mul(ctxs, psw, rz)
            nc.sync.dma_start(out=ctx_out[b:b + 1, :], in_=ctxs)

        cur = fv_nat0
        for b in range(BLOC):
            cur = main_stage(b, cur)
            if (b - 1) in done:
                ws_stage(b - 1)
        # flush: finish the final row's softmax from the partial gather
        st = state.pop(BLOC - 1)
        e_t, mrow1 = st["e_t"], st["mrow1"]
        nc.sync.dma_start(
            out=e_t[:, 24:32],
            in_=bass.AP(tensor=st["e_d"].tensor,
                        offset=st["e_d"].offset + 24,
                        ap=[[NL, 128], [1, 8]]))
        mrow = small.tile([128, 1], f32, tag="mrow", name="mrow")
        nc.vector.reduce_max(out=mrow, in_=e_t[:, 24:32],
                             axis=mybir.AxisListType.X)
        mcmb = small.tile([128, 1], f32, tag="mall", name="mcmb")
        nc.vector.scalar_tensor_tensor(out=mcmb, in0=mrow1, scalar=1.0,
                                       in1=mrow, op0=mybir.AluOpType.mult,
                                       op1=mybir.AluOpType.max)
        mall = small.tile([128, 1], f32, tag="mrow", name="mall")
        nc.gpsimd.partition_all_reduce(mall, mcmb, channels=128,
                                       reduce_op=bass_isa.ReduceOp.max)
        negm = small.tile([128, 1], f32, tag="negm", name="negm")
        nc.vector.tensor_scalar_mul(negm, mall, -1.0)
        p_t = small.tile([128, NL], f16, tag="p_t", name="p_t")
        srow = small.tile([128, 1], f32, tag="srow", name="srow")
        nc.scalar.activation(out=p_t, in_=e_t, func=AF.Exp, bias=negm,
                             scale=1.0, accum_out=srow)
        sall = small.tile([128, 1], f32, tag="sall", name="sall")
        nc.gpsimd.partition_all_reduce(sall, srow, channels=128,
                                       reduce_op=bass_isa.ReduceOp.add)
        rz = small.tile([1, 1], f32, tag="rz", name="rz")
        nc.vector.reciprocal(out=rz, in_=sall[0:1, :])
        done[BLOC - 1] = (st["fv_nat"], p_t, rz)
        ws_stage(BLOC - 1)

    nc.compile()
    return nc


def _get_nc():
    if "nc" not in _CACHE:
        _CACHE["nc"] = _build()
    return _CACHE["nc"]


def kernel(hidden_state, feature_vectors, Wa, Ua, w, ba):
    from concourse.bass_utils import run_bass_kernel_spmd

    nc = _get_nc()
    hidden_state = np.ascontiguousarray(hidden_state, dtype=np.float32)
    feature_vectors = np.ascontiguousarray(feature_vectors, dtype=np.float32)
    params = {
        "Wa": np.ascontiguousarray(Wa, dtype=np.float32),
        "Ua": np.ascontiguousarray(Ua, dtype=np.float32),
        "w": np.ascontiguousarray(w, dtype=np.float32),
        "ba": np.ascontiguousarray(ba, dtype=np.float32),
    }
    in_maps = [
        {
            "hidden_state": hidden_state[c * BLOC:(c + 1) * BLOC],
            "feature_vectors": feature_vectors[c * BLOC:(c + 1) * BLOC],
            **params,
        }
        for c in range(NCORES)
    ]
    res = run_bass_kernel_spmd(nc, in_maps, list(range(NCORES)))
    return np.concatenate([res.results[c]["context"] for c in range(NCORES)],
                          axis=0)


# revision 39
# speedup vs baseline: 1.0717x; 1.0717x over previous
"""Bass/Tile Trainium2 kernel for the additive-attention module.

reference (per batch row b):
    q = hidden_state @ Wa.T + ba                 # [A]
    k = feature_vectors[b] @ Ua.T                # [L, A]
    e = tanh(q + k) @ w                          # [L]
    attn = softmax(e)                            # [L]
    context[b] = attn @ feature_vectors[b]       # [M]

Sharding: data-parallel over batch B=64 -> 8 cores x 8 rows, params
replicated, no collectives. Each core streams its 32 MB feature_vector
shard from HBM exactly once.

Precision: fp32 matmuls cost 4 cycles/column on TRN2 PE; fp16 costs 1.
The fv pipeline (fv, Ua, tanh output, attn weights) runs in fp16;
softmax statistics and all accumulations (PSUM) stay fp32.

Per-core dataflow (per batch row):
  - fv cast fp32->fp16 during the HBM DMA (SWDGE), natural [l, m] layout
  - PE transposes 128x128 fp16 tiles of fv into [m, l] layout (PSUM),
    DVE evacuates; k-matmul streams fv.T columns with Ua.T stationary
    (XBAR dma transpose measured 16us/row + full serialization against
    the SWDGE loads, so the PE identity-transpose is the right path)
  - ScalarE evacuates the k PSUM with fused per-partition bias q[a] and
    tanh in one ACTIVATE (fp16 out)
  - e = w.T @ tanh(...) on PE; softmax: DRAM-bounce reshape of e
    [1,4096] -> [128,32], DVE row max, GPSIMD cross-partition max, ACT
    exp with accum_out row sums, GPSIMD cross-partition sum
  - weighted sum on PE: attn column [128,1] fp16 stationary, natural
    fv tiles streaming; denominator applied at the end in fp32
  - queue discipline: fv loads get the GPSIMD SWDGE queue (with the
    cross-partition reduces, emitted only when nearly ready), e bounce +
    ctx stores ride Sync, param prologue rides the Scalar HWDGE queue
  - row b's softmax tail is split and deferred into row b+1's emission
    (gather+max at the 3rd k-group, exp at the very end) and its
    weighted sum after row b+1's main stage, so neither PE nor any
    in-order queue ever waits on the softmax dependency chain
"""

import numpy as np

B, R, M, A, L = 64, 512, 256, 256, 4096
NCORES = 8
BLOC = B // NCORES  # 8 batch rows per core
NL = L // 128  # 32 l-chunks of 128
NJG = 8  # j-groups of 512 l-columns
JW = L // NJG  # 512

_CACHE = {}


def _build():
    from contextlib import ExitStack

    import concourse.bacc as bacc
    import concourse.bass as bass
    import concourse.bass_isa as bass_isa
    import concourse.mybir as mybir
    import concourse.tile as tile
    from concourse.masks import make_identity

    f32 = mybir.dt.float32
    f16 = mybir.dt.float16
    AF = mybir.ActivationFunctionType

    nc = bacc.Bacc("TRN2", target_bir_lowering=False, debug=False,
                   num_devices=NCORES)

    hs = nc.dram_tensor("hidden_state", [BLOC, R], f32, kind="ExternalInput").ap()
    fv = nc.dram_tensor("feature_vectors", [BLOC, L, M], f32,
                        kind="ExternalInput").ap()
    Wa = nc.dram_tensor("Wa", [A, R], f32, kind="ExternalInput").ap()
    Ua = nc.dram_tensor("Ua", [A, M], f32, kind="ExternalInput").ap()
    w = nc.dram_tensor("w", [A, 1], f32, kind="ExternalInput").ap()
    ba = nc.dram_tensor("ba", [1, A], f32, kind="ExternalInput").ap()
    ctx_out = nc.dram_tensor("context", [BLOC, M], f32, kind="ExternalOutput").ap()

    with tile.TileContext(nc) as tc, ExitStack() as ctx:
        singles = ctx.enter_context(tc.tile_pool(name="singles", bufs=1))
        ldpool = ctx.enter_context(tc.tile_pool(name="ldpool", bufs=2))
        fvpool = ctx.enter_context(tc.tile_pool(name="fvpool", bufs=3))
        work = ctx.enter_context(tc.tile_pool(name="work", bufs=3))
        small = ctx.enter_context(tc.tile_pool(name="small", bufs=2))
        ps_tp = ctx.enter_context(tc.tile_pool(name="ps_tp", bufs=2, space="PSUM"))
        ps_k = ctx.enter_context(tc.tile_pool(name="ps_k", bufs=3, space="PSUM"))
        ps_e = ctx.enter_context(tc.tile_pool(name="ps_e", bufs=2, space="PSUM"))
        ps_mm = ctx.enter_context(tc.tile_pool(name="ps_mm", bufs=1, space="PSUM"))
        dram = ctx.enter_context(tc.tile_pool(name="dram", bufs=2, space="DRAM"))

        # identities first: they ride the GPSIMD queue and gate every
        # transpose, so they must precede the fv load issues there
        ident = singles.tile([128, 128], f32, tag="ident", name="ident")
        make_identity(nc, ident)
        ident16 = singles.tile([128, 128], f16, tag="ident16", name="ident16")
        make_identity(nc, ident16)

        # batch 0's fv cast-load is issued split in four so the prologue
        # transposes chase the arriving 1 MB chunks; steady-state loads
        # stay whole
        fv_nat0 = fvpool.tile([128, NL, M], f16, tag="fv", name="fv")
        for part in range(4):
            fsrc = bass.AP(tensor=fv.tensor, offset=part * (NL // 4) * 128 * M,
                           ap=[[M, 128], [128 * M, NL // 4], [1, M]])
            nc.gpsimd.dma_start(
                out=fv_nat0[:, part * (NL // 4):(part + 1) * (NL // 4), :],
                in_=fsrc)

        # ---- parameters into contraction-major layouts ----
        # (param DMAs ride the Scalar HWDGE queue; Sync keeps the e bounce,
        # GPSIMD keeps the fv loads)
        # WaT[rt] [128(r), 256(a)] fp32: WaT[rt][k, a] = Wa[a, 128*rt + k]
        WaT = [singles.tile([128, A], f32, tag=f"WaT{rt}", name=f"WaT{rt}")
               for rt in range(4)]
        for at in range(2):
            wa_nat = ldpool.tile([128, R], f32, tag="ld", name="ld")
            nc.scalar.dma_start(out=wa_nat, in_=Wa[at * 128:(at + 1) * 128, :])
            for rt in range(4):
                ps = ps_mm.tile([128, 128], f32, tag="mm", name="mm")
                nc.tensor.transpose(ps, wa_nat[:, rt * 128:(rt + 1) * 128], ident)
                nc.vector.tensor_copy(out=WaT[rt][:, at * 128:(at + 1) * 128],
                                      in_=ps)
        # UaT[mh] [128(m), 256(a)] fp16: UaT[mh][k, a] = Ua[a, 128*mh + k]
        UaT = [singles.tile([128, A], f16, tag=f"UaT{mh}", name=f"UaT{mh}")
               for mh in range(2)]
        for at in range(2):
            ua_nat = ldpool.tile([128, M], f32, tag="ld", name="ld")
            nc.scalar.dma_start(out=ua_nat, in_=Ua[at * 128:(at + 1) * 128, :])
            for mh in range(2):
                ps = ps_mm.tile([128, 128], f32, tag="mm", name="mm")
                nc.tensor.transpose(ps, ua_nat[:, mh * 128:(mh + 1) * 128], ident)
                nc.vector.tensor_copy(out=UaT[mh][:, at * 128:(at + 1) * 128],
                                      in_=ps)
        # w as fp16 stationary columns [128, 1] per a-half (cast during DMA)
        w_sb = [singles.tile([128, 1], f16, tag=f"w{ah}", name=f"w{ah}")
                for ah in range(2)]
        for ah in range(2):
            nc.gpsimd.dma_start(out=w_sb[ah], in_=w[ah * 128:(ah + 1) * 128, :])

        # hsT[rt] [128(r), BLOC] fp32 — the strided gather rides the GPSIMD
        # SWDGE queue (cheap descriptor gen there; on the Scalar HWDGE
        # queue it would block ba/ua for ~14us)
        hsT = [singles.tile([128, BLOC], f32, tag=f"hsT{rt}", name=f"hsT{rt}")
               for rt in range(4)]
        for rt in range(4):
            src = bass.AP(tensor=hs.tensor, offset=rt * 128,
                          ap=[[1, 128], [R, BLOC]])
            nc.gpsimd.dma_start(out=hsT[rt], in_=src)

        # q = hs @ Wa.T + ba   -> [BLOC, A] fp32
        ba_b = singles.tile([BLOC, A], f32, tag="ba", name="ba")
        nc.scalar.dma_start(out=ba_b,
                            in_=bass.AP(tensor=ba.tensor, offset=0,
                                        ap=[[0, BLOC], [1, A]]))
        q_ps = ps_mm.tile([BLOC, A], f32, tag="mm", name="mm")
        for rt in range(4):
            nc.tensor.matmul(q_ps, lhsT=hsT[rt], rhs=WaT[rt],
                             start=(rt == 0), stop=(rt == 3))
        q_sb = singles.tile([BLOC, A], f32, tag="q", name="q")
        nc.vector.tensor_add(q_sb, q_ps, ba_b)
        # qT[ah] [128(a), BLOC] fp32
        qT = [singles.tile([128, BLOC], f32, tag=f"qT{ah}", name=f"qT{ah}")
              for ah in range(2)]
        for ah in range(2):
            ps = ps_mm.tile([128, BLOC], f32, tag="mm", name="mm")
            nc.tensor.transpose(ps, q_sb[:, ah * 128:(ah + 1) * 128],
                                ident[:BLOC, :BLOC])
            nc.vector.tensor_copy(out=qT[ah], in_=ps)

        # ---- deferred softmax tail machinery ----
        state = {}  # b -> dict with per-row tiles awaiting tail pieces
        done = {}   # b -> (fv_nat, p_t, rz) awaiting weighted sum

        def tail_gather(b):
            """e bounce-back DMA for row b (emitted early in row b+1: the
            e_d writes of row b are complete by then). The max reduction
            is deferred to tail_max so a late gather never blocks the
            in-order DVE queue in front of the fvT evacuation copies."""
            st = state[b]
            e_t = small.tile([128, NL], f32, tag="e_t", name="e_t")
            nc.sync.dma_start(
                out=e_t,
                in_=bass.AP(tensor=st["e_d"].tensor, offset=st["e_d"].offset,
                            ap=[[1, 128], [128, NL]]))
            st["e_t"] = e_t

        def tail_max(b):
            st = state[b]
            mrow = small.tile([128, 1], f32, tag="mrow", name="mrow")
            nc.vector.reduce_max(out=mrow, in_=st["e_t"],
                                 axis=mybir.AxisListType.X)
            mall = small.tile([128, 1], f32, tag="mall", name="mall")
            nc.gpsimd.partition_all_reduce(mall, mrow, channels=128,
                                           reduce_op=bass_isa.ReduceOp.max)
            negm = small.tile([128, 1], f32, tag="negm", name="negm")
            nc.vector.tensor_scalar_mul(negm, mall, -1.0)
            st["negm"] = negm

        def tail_exp(b):
            """exp + sum for row b (emitted at the end of row b+1's main
            stage so it never blocks row b+1's tanh on the ACT queue)."""
            st = state.pop(b)
            p_t = small.tile([128, NL], f16, tag="p_t", name="p_t")
            srow = small.tile([128, 1], f32, tag="srow", name="srow")
            nc.scalar.activation(out=p_t, in_=st["e_t"], func=AF.Exp,
                                 bias=st["negm"], scale=1.0, accum_out=srow)
            sall = small.tile([128, 1], f32, tag="sall", name="sall")
            nc.gpsimd.partition_all_reduce(sall, srow, channels=128,
                                           reduce_op=bass_isa.ReduceOp.add)
            rz = small.tile([1, 1], f32, tag="rz", name="rz")
            nc.vector.reciprocal(out=rz, in_=sall[0:1, :])
            done[b] = (st["fv_nat"], p_t, rz)

        # ---- main per-batch-row pipeline ----
        def main_stage(b, fv_nat):
            # next row's load is issued first so the GPSIMD queue never
            # sits behind the cross-partition reduces
            if b + 1 < BLOC:
                fv_nat_n = fvpool.tile([128, NL, M], f16, tag="fv", name="fv")
                src = bass.AP(tensor=fv.tensor, offset=(b + 1) * L * M,
                              ap=[[M, 128], [128 * M, NL], [1, M]])
                nc.gpsimd.dma_start(out=fv_nat_n, in_=src)
            else:
                fv_nat_n = None

            e_sb = small.tile([1, L], f32, tag="e_sb", name="e_sb")
            e_d = dram.tile([L], f32, tag="e_d", name="e_d")

            # software-pipelined j-groups: PE emits transposes(i),
            # k-matmuls(i-1), e-matmul(i-2) per step so it never stalls on
            # the DVE fvT-copy or ACT tanh of the current group.
            fvT_q = {}
            t_q = {}

            def emit_T(jg):
                # both m-halves' transposes share one fp16 PSUM bank;
                # alternate the mh target region between consecutive
                # transposes so their weight loads can pipeline
                pst = ps_tp.tile([128, 2, JW], f16, tag="tp", name="tp")
                for c in range(4):
                    t = jg * 4 + c
                    for mh in range(2):
                        nc.tensor.transpose(
                            pst[:, mh, c * 128:(c + 1) * 128],
                            fv_nat[:, t, mh * 128:(mh + 1) * 128], ident16)
                fvT = [work.tile([128, JW], f16, tag=f"fvT{mh}", name=f"fvT{mh}")
                       for mh in range(2)]
                for mh in range(2):
                    nc.vector.tensor_copy(out=fvT[mh], in_=pst[:, mh, :])
                fvT_q[jg] = fvT

            def emit_K(jg):
                # the two ah accumulation groups are interleaved so each
                # matmul's weight load overlaps the other group's stream
                # (within one group the mh0->mh1 pair is strictly serial)
                fvT = fvT_q.pop(jg)
                t_sb = [work.tile([128, JW], f16, tag=f"t{ah}", name=f"t{ah}")
                        for ah in range(2)]
                psk = [ps_k.tile([128, JW], f32, tag="kk", name="kk")
                       for _ in range(2)]
                for mh in range(2):
                    for ah in range(2):
                        nc.tensor.matmul(
                            psk[ah], lhsT=UaT[mh][:, ah * 128:(ah + 1) * 128],
                            rhs=fvT[mh], start=(mh == 0), stop=(mh == 1))
                for ah in range(2):
                    nc.scalar.activation(out=t_sb[ah], in_=psk[ah],
                                         func=AF.Tanh,
                                         bias=qT[ah][:, b:b + 1], scale=1.0)
                t_q[jg] = t_sb

            def emit_E(jg):
                t_sb = t_q.pop(jg)
                pse = ps_e.tile([1, JW], f32, tag="ee", name="ee")
                for ah in range(2):
                    nc.tensor.matmul(pse, lhsT=w_sb[ah], rhs=t_sb[ah],
                                     start=(ah == 0), stop=(ah == 1))
                nc.vector.tensor_copy(out=e_sb[:, jg * JW:(jg + 1) * JW],
                                      in_=pse)
                nc.sync.dma_start(
                    out=bass.AP(tensor=e_d.tensor, offset=e_d.offset + jg * JW,
                                ap=[[0, 1], [1, JW]]),
                    in_=e_sb[:, jg * JW:(jg + 1) * JW])

            part = {}
            for i in range(NJG + 2):
                if i < NJG:
                    emit_T(i)
                if i == 3 and (b - 1) in state:
                    tail_gather(b - 1)
                if 1 <= i <= NJG:
                    emit_K(i - 1)
                if 2 <= i:
                    emit_E(i - 2)
                if b == BLOC - 1 and i == NJG - 1:
                    # final row: gather + partial max over chunks 0..23
                    # while j-groups 6..7 still compute, shortening the
                    # end-of-kernel softmax chain
                    e_t = small.tile([128, NL], f32, tag="e_t", name="e_t")
                    nc.sync.dma_start(
                        out=e_t[:, 0:24],
                        in_=bass.AP(tensor=e_d.tensor, offset=e_d.offset,
                                    ap=[[1, 128], [128, 24]]))
                    mrow1 = small.tile([128, 1], f32, tag="mall",
                                       name="mrow1")
                    nc.vector.reduce_max(out=mrow1, in_=e_t[:, 0:24],
                                         axis=mybir.AxisListType.X)
                    part = {"e_t": e_t, "mrow1": mrow1}

            state[b] = {"fv_nat": fv_nat, "e_d": e_d, **part}
            if (b - 1) in state:
                tail_max(b - 1)
                tail_exp(b - 1)
            return fv_nat_n

        # weighted sum, one batch row behind
        def ws_stage(b):
            fv_nat, p_t, rz = done.pop(b)
            psw = ps_mm.tile([1, M], f32, tag="mm", name="mm")
            for t in range(NL):
                nc.tensor.matmul(psw, lhsT=p_t[:, t:t + 1], rhs=fv_nat[:, t, :],
                                 start=(t == 0), stop=(t == NL - 1))
            ctxs = small.tile([1, M], f32, tag="ctx", name="ctx")
            nc.vector.tensor_scalar_mul(ctxs, psw, rz)
            nc.sync.dma_start(out=ctx_out[b:b + 1, :], in_=ctxs)

        cur = fv_nat0
        for b in range(BLOC):
            cur = main_stage(b, cur)
            if (b - 1) in done:
                ws_stage(b - 1)
        # flush: finish the final row's softmax from the partial gather
        st = state.pop(BLOC - 1)
        e_t, mrow1 = st["e_t"], st["mrow1"]
        nc.sync.dma_start(
            out=e_t[:, 24:32],
            in_=bass.AP(tensor=st["e_d"].tensor,
                        offset=st["e_d"].offset + 24 * 128,
                        ap=[[1, 128], [128, 8]]))
        mrow = small.tile([128, 1], f32, tag="mrow", name="mrow")
        nc.vector.reduce_max(out=mrow, in_=e_t[:, 24:32],
                             axis=mybir.AxisListType.X)
        mcmb = small.tile([128, 1], f32, tag="mall", name="mcmb")
        nc.vector.scalar_tensor_tensor(out=mcmb, in0=mrow1, scalar=1.0,
                                       in1=mrow, op0=mybir.AluOpType.mult,
                                       op1=mybir.AluOpType.max)
        mall = small.tile([128, 1], f32, tag="mrow", name="mall")
        nc.gpsimd.partition_all_reduce(mall, mcmb, channels=128,
                                       reduce_op=bass_isa.ReduceOp.max)
        negm = small.tile([128, 1], f32, tag="negm", name="negm")
        nc.vector.tensor_scalar_mul(negm, mall, -1.0)
        p_t = small.tile([128, NL], f16, tag="p_t", name="p_t")
        srow = small.tile([128, 1], f32, tag="srow", name="srow")
        nc.scalar.activation(out=p_t, in_=e_t, func=AF.Exp, bias=negm,
                             scale=1.0, accum_out=srow)
        sall = small.tile([128, 1], f32, tag="sall", name="sall")
        nc.gpsimd.partition_all_reduce(sall, srow, channels=128,
                                       reduce_op=bass_isa.ReduceOp.add)
        rz = small.tile([1, 1], f32, tag="rz", name="rz")
        nc.vector.reciprocal(out=rz, in_=sall[0:1, :])
        done[BLOC - 1] = (st["fv_nat"], p_t, rz)
        ws_stage(BLOC - 1)

    nc.compile()
    return nc


def _get_nc():
    if "nc" not in _CACHE:
        _CACHE["nc"] = _build()
    return _CACHE["nc"]


def kernel(hidden_state, feature_vectors, Wa, Ua, w, ba):
    from concourse.bass_utils import run_bass_kernel_spmd

    nc = _get_nc()
    hidden_state = np.ascontiguousarray(hidden_state, dtype=np.float32)
    feature_vectors = np.ascontiguousarray(feature_vectors, dtype=np.float32)
    params = {
        "Wa": np.ascontiguousarray(Wa, dtype=np.float32),
        "Ua": np.ascontiguousarray(Ua, dtype=np.float32),
        "w": np.ascontiguousarray(w, dtype=np.float32),
        "ba": np.ascontiguousarray(ba, dtype=np.float32),
    }
    in_maps = [
        {
            "hidden_state": hidden_state[c * BLOC:(c + 1) * BLOC],
            "feature_vectors": feature_vectors[c * BLOC:(c + 1) * BLOC],
            **params,
        }
        for c in range(NCORES)
    ]
    res = run_bass_kernel_spmd(nc, in_maps, list(range(NCORES)))
    return np.concatenate([res.results[c]["context"] for c in range(NCORES)],
                          axis=0)


# revision 41
# speedup vs baseline: 1.2134x; 1.1322x over previous
"""Bass/Tile Trainium2 kernel for the additive-attention module.

reference (per batch row b):
    q = hidden_state @ Wa.T + ba                 # [A]
    k = feature_vectors[b] @ Ua.T                # [L, A]
    e = tanh(q + k) @ w                          # [L]
    attn = softmax(e)                            # [L]
    context[b] = attn @ feature_vectors[b]       # [M]

Sharding: data-parallel over batch B=64 -> 8 cores x 8 rows, params
replicated, no collectives. Each core streams its 32 MB feature_vector
shard from HBM exactly once.

Precision: fp32 matmuls cost 4 cycles/column on TRN2 PE; fp16 costs 1.
The fv pipeline (fv, Ua, tanh output, attn weights) runs in fp16;
softmax statistics and all accumulations (PSUM) stay fp32.

Per-core dataflow (per batch row):
  - fv cast fp32->fp16 during the HBM DMA (SWDGE), natural [l, m] layout
  - PE transposes 128x128 fp16 tiles of fv into [m, l] layout (PSUM),
    DVE evacuates; k-matmul streams fv.T columns with Ua.T stationary
    (XBAR dma transpose measured 16us/row + full serialization against
    the SWDGE loads, so the PE identity-transpose is the right path)
  - ScalarE evacuates the k PSUM with fused per-partition bias q[a] and
    tanh in one ACTIVATE (fp16 out)
  - e = w.T @ tanh(...) on PE; softmax: DRAM-bounce reshape of e
    [1,4096] -> [128,32], DVE row max, GPSIMD cross-partition max, ACT
    exp with accum_out row sums, GPSIMD cross-partition sum
  - weighted sum on PE: attn column [128,1] fp16 stationary, natural
    fv tiles streaming; denominator applied at the end in fp32
  - queue discipline: fv loads get the GPSIMD SWDGE queue (with the
    cross-partition reduces, emitted only when nearly ready), e bounce +
    ctx stores ride Sync, param prologue rides the Scalar HWDGE queue
  - row b's softmax tail is split and deferred into row b+1's emission
    (gather+max at the 3rd k-group, exp at the very end) and its
    weighted sum after row b+1's main stage, so neither PE nor any
    in-order queue ever waits on the softmax dependency chain
"""

import numpy as np

B, R, M, A, L = 64, 512, 256, 256, 4096
NCORES = 8
BLOC = B // NCORES  # 8 batch rows per core
NL = L // 128  # 32 l-chunks of 128
NJG = 8  # j-groups of 512 l-columns
JW = L // NJG  # 512

_CACHE = {}


def _build():
    from contextlib import ExitStack

    import concourse.bacc as bacc
    import concourse.bass as bass
    import concourse.bass_isa as bass_isa
    import concourse.mybir as mybir
    import concourse.tile as tile
    from concourse.masks import make_identity

    f32 = mybir.dt.float32
    f16 = mybir.dt.float16
    AF = mybir.ActivationFunctionType

    nc = bacc.Bacc("TRN2", target_bir_lowering=False, debug=False,
                   num_devices=NCORES)

    hs = nc.dram_tensor("hidden_state", [BLOC, R], f32, kind="ExternalInput").ap()
    fv = nc.dram_tensor("feature_vectors", [BLOC, L, M], f32,
                        kind="ExternalInput").ap()
    Wa = nc.dram_tensor("Wa", [A, R], f32, kind="ExternalInput").ap()
    Ua = nc.dram_tensor("Ua", [A, M], f32, kind="ExternalInput").ap()
    w = nc.dram_tensor("w", [A, 1], f32, kind="ExternalInput").ap()
    ba = nc.dram_tensor("ba", [1, A], f32, kind="ExternalInput").ap()
    ctx_out = nc.dram_tensor("context", [BLOC, M], f32, kind="ExternalOutput").ap()

    with tile.TileContext(nc) as tc, ExitStack() as ctx:
        singles = ctx.enter_context(tc.tile_pool(name="singles", bufs=1))
        ldpool = ctx.enter_context(tc.tile_pool(name="ldpool", bufs=2))
        fvpool = ctx.enter_context(tc.tile_pool(name="fvpool", bufs=3))
        work = ctx.enter_context(tc.tile_pool(name="work", bufs=3))
        small = ctx.enter_context(tc.tile_pool(name="small", bufs=2))
        ps_tp = ctx.enter_context(tc.tile_pool(name="ps_tp", bufs=2, space="PSUM"))
        ps_k = ctx.enter_context(tc.tile_pool(name="ps_k", bufs=3, space="PSUM"))
        ps_e = ctx.enter_context(tc.tile_pool(name="ps_e", bufs=2, space="PSUM"))
        ps_mm = ctx.enter_context(tc.tile_pool(name="ps_mm", bufs=1, space="PSUM"))
        dram = ctx.enter_context(tc.tile_pool(name="dram", bufs=2, space="DRAM"))

        # identities first: they ride the GPSIMD queue and gate every
        # transpose, so they must precede the fv load issues there
        ident = singles.tile([128, 128], f32, tag="ident", name="ident")
        make_identity(nc, ident)
        ident16 = singles.tile([128, 128], f16, tag="ident16", name="ident16")
        make_identity(nc, ident16)

        # batch 0's fv cast-load is issued split in four so the prologue
        # transposes chase the arriving 1 MB chunks; steady-state loads
        # stay whole
        fv_nat0 = fvpool.tile([128, NL, M], f16, tag="fv", name="fv")
        for part in range(4):
            fsrc = bass.AP(tensor=fv.tensor, offset=part * (NL // 4) * 128 * M,
                           ap=[[M, 128], [128 * M, NL // 4], [1, M]])
            nc.gpsimd.dma_start(
                out=fv_nat0[:, part * (NL // 4):(part + 1) * (NL // 4), :],
                in_=fsrc)

        # ---- parameters into contraction-major layouts ----
        # (param DMAs ride the Scalar HWDGE queue; Sync keeps the e bounce,
        # GPSIMD keeps the fv loads)
        # WaT[rt] [128(r), 256(a)] fp32: WaT[rt][k, a] = Wa[a, 128*rt + k]
        WaT = [singles.tile([128, A], f32, tag=f"WaT{rt}", name=f"WaT{rt}")
               for rt in range(4)]
        for at in range(2):
            wa_nat = ldpool.tile([128, R], f32, tag="ld", name="ld")
            nc.scalar.dma_start(out=wa_nat, in_=Wa[at * 128:(at + 1) * 128, :])
            for rt in range(4):
                ps = ps_mm.tile([128, 128], f32, tag="mm", name="mm")
                nc.tensor.transpose(ps, wa_nat[:, rt * 128:(rt + 1) * 128], ident)
                nc.vector.tensor_copy(out=WaT[rt][:, at * 128:(at + 1) * 128],
                                      in_=ps)
        # UaT[mh] [128(m), 256(a)] fp16: UaT[mh][k, a] = Ua[a, 128*mh + k]
        UaT = [singles.tile([128, A], f16, tag=f"UaT{mh}", name=f"UaT{mh}")
               for mh in range(2)]
        for at in range(2):
            ua_nat = ldpool.tile([128, M], f32, tag="ld", name="ld")
            nc.scalar.dma_start(out=ua_nat, in_=Ua[at * 128:(at + 1) * 128, :])
            for mh in range(2):
                ps = ps_mm.tile([128, 128], f32, tag="mm", name="mm")
                nc.tensor.transpose(ps, ua_nat[:, mh * 128:(mh + 1) * 128], ident)
                nc.vector.tensor_copy(out=UaT[mh][:, at * 128:(at + 1) * 128],
                                      in_=ps)
        # w as fp16 stationary columns [128, 1] per a-half (cast during DMA)
        w_sb = [singles.tile([128, 1], f16, tag=f"w{ah}", name=f"w{ah}")
                for ah in range(2)]
        for ah in range(2):
            nc.gpsimd.dma_start(out=w_sb[ah], in_=w[ah * 128:(ah + 1) * 128, :])

        # hsT[rt] [128(r), BLOC] fp32 — the strided gather rides the GPSIMD
        # SWDGE queue (cheap descriptor gen there; on the Scalar HWDGE
        # queue it would block ba/ua for ~14us)
        hsT = [singles.tile([128, BLOC], f32, tag=f"hsT{rt}", name=f"hsT{rt}")
               for rt in range(4)]
        for rt in range(4):
            src = bass.AP(tensor=hs.tensor, offset=rt * 128,
                          ap=[[1, 128], [R, BLOC]])
            nc.gpsimd.dma_start(out=hsT[rt], in_=src)

        # q = hs @ Wa.T + ba   -> [BLOC, A] fp32
        ba_b = singles.tile([BLOC, A], f32, tag="ba", name="ba")
        nc.scalar.dma_start(out=ba_b,
                            in_=bass.AP(tensor=ba.tensor, offset=0,
                                        ap=[[0, BLOC], [1, A]]))
        q_ps = ps_mm.tile([BLOC, A], f32, tag="mm", name="mm")
        for rt in range(4):
            nc.tensor.matmul(q_ps, lhsT=hsT[rt], rhs=WaT[rt],
                             start=(rt == 0), stop=(rt == 3))
        q_sb = singles.tile([BLOC, A], f32, tag="q", name="q")
        nc.vector.tensor_add(q_sb, q_ps, ba_b)
        # qT[ah] [128(a), BLOC] fp32
        qT = [singles.tile([128, BLOC], f32, tag=f"qT{ah}", name=f"qT{ah}")
              for ah in range(2)]
        for ah in range(2):
            ps = ps_mm.tile([128, BLOC], f32, tag="mm", name="mm")
            nc.tensor.transpose(ps, q_sb[:, ah * 128:(ah + 1) * 128],
                                ident[:BLOC, :BLOC])
            nc.vector.tensor_copy(out=qT[ah], in_=ps)

        # ---- deferred softmax tail machinery ----
        state = {}  # b -> dict with per-row tiles awaiting tail pieces
        done = {}   # b -> (fv_nat, p_t, rz) awaiting weighted sum

        def tail_gather(b):
            """e bounce-back DMA for row b (emitted early in row b+1: the
            e_d writes of row b are complete by then). The max reduction
            is deferred to tail_max so a late gather never blocks the
            in-order DVE queue in front of the fvT evacuation copies."""
            st = state[b]
            e_t = small.tile([128, NL], f32, tag="e_t", name="e_t")
            # split the 4096-descriptor partition-scatter gather across
            # both HWDGE queues (~1.7ns/descriptor of queue-side gen) so
            # it completes in half the time; the Scalar piece has no
            # unsatisfied waits at dispatch so it cannot stall the tanhs
            nc.sync.dma_start(
                out=e_t[:, 0:NL // 2],
                in_=bass.AP(tensor=st["e_d"].tensor, offset=st["e_d"].offset,
                            ap=[[1, 128], [128, NL // 2]]))
            nc.scalar.dma_start(
                out=e_t[:, NL // 2:],
                in_=bass.AP(tensor=st["e_d"].tensor,
                            offset=st["e_d"].offset + (NL // 2) * 128,
                            ap=[[1, 128], [128, NL // 2]]))
            st["e_t"] = e_t

        def tail_max(b):
            st = state[b]
            mrow = small.tile([128, 1], f32, tag="mrow", name="mrow")
            nc.vector.reduce_max(out=mrow, in_=st["e_t"],
                                 axis=mybir.AxisListType.X)
            mall = small.tile([128, 1], f32, tag="mall", name="mall")
            nc.gpsimd.partition_all_reduce(mall, mrow, channels=128,
                                           reduce_op=bass_isa.ReduceOp.max)
            negm = small.tile([128, 1], f32, tag="negm", name="negm")
            nc.vector.tensor_scalar_mul(negm, mall, -1.0)
            st["negm"] = negm

        def tail_exp(b):
            """exp + sum for row b (emitted at the end of row b+1's main
            stage so it never blocks row b+1's tanh on the ACT queue)."""
            st = state.pop(b)
            p_t = small.tile([128, NL], f16, tag="p_t", name="p_t")
            srow = small.tile([128, 1], f32, tag="srow", name="srow")
            nc.scalar.activation(out=p_t, in_=st["e_t"], func=AF.Exp,
                                 bias=st["negm"], scale=1.0, accum_out=srow)
            sall = small.tile([128, 1], f32, tag="sall", name="sall")
            nc.gpsimd.partition_all_reduce(sall, srow, channels=128,
                                           reduce_op=bass_isa.ReduceOp.add)
            rz = small.tile([1, 1], f32, tag="rz", name="rz")
            nc.vector.reciprocal(out=rz, in_=sall[0:1, :])
            done[b] = (st["fv_nat"], p_t, rz)

        # ---- main per-batch-row pipeline ----
        def main_stage(b, fv_nat):
            # next row's load is issued first so the GPSIMD queue never
            # sits behind the cross-partition reduces
            if b + 1 < BLOC:
                fv_nat_n = fvpool.tile([128, NL, M], f16, tag="fv", name="fv")
                src = bass.AP(tensor=fv.tensor, offset=(b + 1) * L * M,
                              ap=[[M, 128], [128 * M, NL], [1, M]])
                nc.gpsimd.dma_start(out=fv_nat_n, in_=src)
            else:
                fv_nat_n = None

            e_sb = small.tile([1, L], f32, tag="e_sb", name="e_sb")
            e_d = dram.tile([L], f32, tag="e_d", name="e_d")

            # software-pipelined j-groups: PE emits transposes(i),
            # k-matmuls(i-1), e-matmul(i-2) per step so it never stalls on
            # the DVE fvT-copy or ACT tanh of the current group.
            fvT_q = {}
            t_q = {}

            def emit_T(jg):
                # both m-halves' transposes share one fp16 PSUM bank;
                # alternate the mh target region between consecutive
                # transposes so their weight loads can pipeline
                pst = ps_tp.tile([128, 2, JW], f16, tag="tp", name="tp")
                for c in range(4):
                    t = jg * 4 + c
                    for mh in range(2):
                        nc.tensor.transpose(
                            pst[:, mh, c * 128:(c + 1) * 128],
                            fv_nat[:, t, mh * 128:(mh + 1) * 128], ident16)
                fvT = [work.tile([128, JW], f16, tag=f"fvT{mh}", name=f"fvT{mh}")
                       for mh in range(2)]
                for mh in range(2):
                    nc.vector.tensor_copy(out=fvT[mh], in_=pst[:, mh, :])
                fvT_q[jg] = fvT

            def emit_K(jg):
                # the two ah accumulation groups are interleaved so each
                # matmul's weight load overlaps the other group's stream
                # (within one group the mh0->mh1 pair is strictly serial)
                fvT = fvT_q.pop(jg)
                t_sb = [work.tile([128, JW], f16, tag=f"t{ah}", name=f"t{ah}")
                        for ah in range(2)]
                psk = [ps_k.tile([128, JW], f32, tag="kk", name="kk")
                       for _ in range(2)]
                for mh in range(2):
                    for ah in range(2):
                        nc.tensor.matmul(
                            psk[ah], lhsT=UaT[mh][:, ah * 128:(ah + 1) * 128],
                            rhs=fvT[mh], start=(mh == 0), stop=(mh == 1))
                for ah in range(2):
                    nc.scalar.activation(out=t_sb[ah], in_=psk[ah],
                                         func=AF.Tanh,
                                         bias=qT[ah][:, b:b + 1], scale=1.0)
                t_q[jg] = t_sb

            def emit_E(jg):
                t_sb = t_q.pop(jg)
                pse = ps_e.tile([1, JW], f32, tag="ee", name="ee")
                for ah in range(2):
                    nc.tensor.matmul(pse, lhsT=w_sb[ah], rhs=t_sb[ah],
                                     start=(ah == 0), stop=(ah == 1))
                nc.vector.tensor_copy(out=e_sb[:, jg * JW:(jg + 1) * JW],
                                      in_=pse)
                nc.sync.dma_start(
                    out=bass.AP(tensor=e_d.tensor, offset=e_d.offset + jg * JW,
                                ap=[[0, 1], [1, JW]]),
                    in_=e_sb[:, jg * JW:(jg + 1) * JW])

            part = {}
            for i in range(NJG + 2):
                if i < NJG:
                    emit_T(i)
                if i == 3 and (b - 1) in state:
                    tail_gather(b - 1)
                if 1 <= i <= NJG:
                    emit_K(i - 1)
                if 2 <= i:
                    emit_E(i - 2)
                if b == BLOC - 1 and i == NJG - 1:
                    # final row: gather + partial max over chunks 0..23
                    # while j-groups 6..7 still compute, shortening the
                    # end-of-kernel softmax chain
                    e_t = small.tile([128, NL], f32, tag="e_t", name="e_t")
                    nc.sync.dma_start(
                        out=e_t[:, 0:12],
                        in_=bass.AP(tensor=e_d.tensor, offset=e_d.offset,
                                    ap=[[1, 128], [128, 12]]))
                    nc.scalar.dma_start(
                        out=e_t[:, 12:24],
                        in_=bass.AP(tensor=e_d.tensor,
                                    offset=e_d.offset + 12 * 128,
                                    ap=[[1, 128], [128, 12]]))
                    mrow1 = small.tile([128, 1], f32, tag="mall",
                                       name="mrow1")
                    nc.vector.reduce_max(out=mrow1, in_=e_t[:, 0:24],
                                         axis=mybir.AxisListType.X)
                    part = {"e_t": e_t, "mrow1": mrow1}

            state[b] = {"fv_nat": fv_nat, "e_d": e_d, **part}
            if (b - 1) in state:
                tail_max(b - 1)
                tail_exp(b - 1)
            return fv_nat_n

        # weighted sum, one batch row behind
        def ws_stage(b):
            fv_nat, p_t, rz = done.pop(b)
            psw = ps_mm.tile([1, M], f32, tag="mm", name="mm")
            for t in range(NL):
                nc.tensor.matmul(psw, lhsT=p_t[:, t:t + 1], rhs=fv_nat[:, t, :],
                                 start=(t == 0), stop=(t == NL - 1))
            ctxs = small.tile([1, M], f32, tag="ctx", name="ctx")
            nc.vector.tensor_scalar_mul(ctxs, psw, rz)
            nc.sync.dma_start(out=ctx_out[b:b + 1, :], in_=ctxs)

        cur = fv_nat0
        for b in range(BLOC):
            cur = main_stage(b, cur)
            if (b - 1) in done:
                ws_stage(b - 1)
        # flush: finish the final row's softmax from the partial gather
        st = state.pop(BLOC - 1)
        e_t, mrow1 = st["e_t"], st["mrow1"]
        nc.sync.dma_start(
            out=e_t[:, 24:32],
            in_=bass.AP(tensor=st["e_d"].tensor,
                        offset=st["e_d"].offset + 24 * 128,
                        ap=[[1, 128], [128, 8]]))
        mrow = small.tile([128, 1], f32, tag="mrow", name="mrow")
        nc.vector.reduce_max(out=mrow, in_=e_t[:, 24:32],
                             axis=mybir.AxisListType.X)
        mcmb = small.tile([128, 1], f32, tag="mall", name="mcmb")
        nc.vector.scalar_tensor_tensor(out=mcmb, in0=mrow1, scalar=1.0,
                                       in1=mrow, op0=mybir.AluOpType.mult,
                                       op1=mybir.AluOpType.max)
        mall = small.tile([128, 1], f32, tag="mrow", name="mall")
        nc.gpsimd.partition_all_reduce(mall, mcmb, channels=128,
                                       reduce_op=bass_isa.ReduceOp.max)
        negm = small.tile([128, 1], f32, tag="negm", name="negm")
        nc.vector.tensor_scalar_mul(negm, mall, -1.0)
        p_t = small.tile([128, NL], f16, tag="p_t", name="p_t")
        srow = small.tile([128, 1], f32, tag="srow", name="srow")
        nc.scalar.activation(out=p_t, in_=e_t, func=AF.Exp, bias=negm,
                             scale=1.0, accum_out=srow)
        sall = small.tile([128, 1], f32, tag="sall", name="sall")
        nc.gpsimd.partition_all_reduce(sall, srow, channels=128,
                                       reduce_op=bass_isa.ReduceOp.add)
        rz = small.tile([1, 1], f32, tag="rz", name="rz")
        nc.vector.reciprocal(out=rz, in_=sall[0:1, :])
        done[BLOC - 1] = (st["fv_nat"], p_t, rz)
        ws_stage(BLOC - 1)

    nc.compile()
    return nc


def _get_nc():
    if "nc" not in _CACHE:
        _CACHE["nc"] = _build()
    return _CACHE["nc"]


def kernel(hidden_state, feature_vectors, Wa, Ua, w, ba):
    from concourse.bass_utils import run_bass_kernel_spmd

    nc = _get_nc()
    hidden_state = np.ascontiguousarray(hidden_state, dtype=np.float32)
    feature_vectors = np.ascontiguousarray(feature_vectors, dtype=np.float32)
    params = {
        "Wa": np.ascontiguousarray(Wa, dtype=np.float32),
        "Ua": np.ascontiguousarray(Ua, dtype=np.float32),
        "w": np.ascontiguousarray(w, dtype=np.float32),
        "ba": np.ascontiguousarray(ba, dtype=np.float32),
    }
    in_maps = [
        {
            "hidden_state": hidden_state[c * BLOC:(c + 1) * BLOC],
            "feature_vectors": feature_vectors[c * BLOC:(c + 1) * BLOC],
            **params,
        }
        for c in range(NCORES)
    ]
    res = run_bass_kernel_spmd(nc, in_maps, list(range(NCORES)))
    return np.concatenate([res.results[c]["context"] for c in range(NCORES)],
                          axis=0)
